# revision 16
# baseline (speedup 1.0000x reference)
"""Causal single-head attention (b=4, n=2048, d=1024, fp32) on 8 TRN2 NeuronCores.

Sharding v2 — uniform padded zig-zag q-split. Core c = (batch c//2, role c%2).
Each role owns 8 of the 16 query subtiles of its batch (zig-zag interleaved,
see ROLE_SUBTILES); every core produces out rows for its own 1024 queries
with the FULL 1024 features.

The SPMD program is identical on all cores; the role only changes host-side
data: which columns land in xTq (own queries), the causal masks, and where
host scatters the output rows. Causal work is padded to the elementwise-max
envelope across the two roles (AV_ENV) so both roles run the same instruction
stream; mask data zeroes the padding.

Per core pipeline (all matmuls bf16 -> fp32 PSUM):
  kT[o,k] = W_k x.T (full 2048 k), qT[o,q] = W_q xq.T (own 1024 q),
  v[k,o] = x W_v.T (full o);  sT[k,q] = scores (contraction over o);
  P = exp(sT/32) * mask (no max subtraction; scaled scores are in [-2.6, 2.6]);
  row sums l[q] via ones-matmul; out[q,o] = (P.T-contraction) / l.
"""

import os
import sys

if os.path.isdir("/opt/trn_rl_repo") and "/opt/trn_rl_repo" not in sys.path:
    sys.path.insert(0, "/opt/trn_rl_repo")

import numpy as np
import ml_dtypes

BF16 = ml_dtypes.bfloat16

B, N, D = 4, 2048, 1024
NCORES = 8
P = 128
QT = 512
NQT = N // QT      # 4 orig q tiles
NKC = N // P       # 16 k chunks
NDC = D // P       # 8 d chunks
NOC = D // P       # 8 o chunks
NQ_OWN = 1024      # own queries per core
SCALE = 1.0 / 32.0

# Zig-zag assignment of the 16 query subtiles (128 rows each) to the two
# roles, chosen so the elementwise-max envelope across roles is minimal:
# slot0 = own subtiles drawn from {0..7}, slot1 from {8..15}.
ROLE_SUBTILES = {
    0: (0, 3, 4, 7, 8, 11, 12, 15),
    1: (1, 2, 5, 6, 9, 10, 13, 14),
}
# k-chunk envelope per (slot, subtile position): max over both roles of the
# causally-needed chunk count for the subtile each role places there.
AV_ENV = ((2, 4, 6, 8), (10, 12, 14, 16))

_CACHE = {}


def _build_module():
    from concourse import bacc
    import concourse.tile as tile
    import concourse.mybir as mybir

    bf = mybir.dt.bfloat16
    f32 = mybir.dt.float32
    Exp = mybir.ActivationFunctionType.Exp

    nc = bacc.Bacc("TRN2", target_bir_lowering=False, debug=False, num_devices=NCORES)

    xT_d = nc.dram_tensor("xT", [D, N], bf, kind="ExternalInput")
    xq_d = nc.dram_tensor("xTq", [D, NQ_OWN], bf, kind="ExternalInput")
    wqT_d = nc.dram_tensor("wqT", [D, D], bf, kind="ExternalInput")
    wkT_d = nc.dram_tensor("wkT", [D, D], bf, kind="ExternalInput")
    wvT_d = nc.dram_tensor("wvT", [D, D], bf, kind="ExternalInput")
    mk_d = nc.dram_tensor("masks", [24, P, QT], bf, kind="ExternalInput")
    out_d = nc.dram_tensor("out", [NQ_OWN, D], f32, kind="ExternalOutput")

    xT_r = xT_d.ap().rearrange("(dc p) n -> p dc n", p=P)
    xq_r = xq_d.ap().rearrange("(dc p) n -> p dc n", p=P)
    wq_r = wqT_d.ap().rearrange("(dc p) o -> p dc o", p=P)
    wk_r = wkT_d.ap().rearrange("(dc p) o -> p dc o", p=P)
    wv_r = wvT_d.ap().rearrange("(dc p) o -> p dc o", p=P)
    mk_r = mk_d.ap().rearrange("j p q -> p j q")
    out_r = out_d.ap().rearrange("(s p) o -> p s o", p=P)

    with tile.TileContext(nc) as tc:
        with tc.tile_pool(name="pers", bufs=1) as pers:
            qT = pers.tile([P, NOC, NQ_OWN], bf, tag="qT")
            kT = pers.tile([P, NOC, N], bf, tag="kT")
            v = pers.tile([P, NKC, D], bf, tag="v")
            mks = pers.tile([P, 24, QT], bf, tag="masks")
            ones = pers.tile([P, 1], bf, tag="ones")

            nc.vector.memset(ones[:], 1.0)

            # PE pre-warm while the first DMAs land (HAM ramp).
            with tc.tile_pool(name="warm", bufs=1, space="PSUM") as warmps:
                wsrc = pers.tile([P, QT], bf, tag="wsrc")
                nc.vector.memset(wsrc[:], 0.0)
                wps = warmps.tile([P, QT], f32, tag="warm")
                for _ in range(10):
                    nc.tensor.matmul(wps, wsrc[:, :P], wsrc[:], start=True, stop=True)

            # ---- projections (K, then Q, then V) ----
            with (
                tc.tile_pool(name="wp", bufs=1) as wp,
                tc.tile_pool(name="xsp", bufs=4) as xsp,
                tc.tile_pool(name="psA", bufs=4, space="PSUM") as psA,
            ):
                wk = wp.tile([P, NDC, D], bf, tag="wk")
                wq = wp.tile([P, NDC, D], bf, tag="wq")
                wv = wp.tile([P, NDC, D], bf, tag="wv")
                xts = []
                for kt in range(NQT):
                    xts.append(xsp.tile([P, NDC, QT], bf, tag="xs", name=f"xk{kt}"))
                # DMA issue order = consumption order.
                for dc in range(NDC):
                    nc.sync.dma_start(wk[:, dc, :], wk_r[:, dc, :])
                    nc.sync.dma_start(xts[0][:, dc, :], xT_r[:, dc, :QT])
                for kt in range(1, NQT):
                    for dc in range(NDC):
                        sl = slice(kt * QT, (kt + 1) * QT)
                        nc.sync.dma_start(xts[kt][:, dc, :], xT_r[:, dc, sl])
                for dc in range(NDC):
                    nc.sync.dma_start(wq[:, dc, :], wq_r[:, dc, :])
                for dc in range(NDC):
                    nc.sync.dma_start(wv[:, dc, :], wv_r[:, dc, :])
                nc.sync.dma_start(mks[:], mk_r)

                # K projection: kT[o, k] (full 2048 k)
                for kt in range(NQT):
                    for oc in range(NOC):
                        ps = psA.tile([P, QT], f32, tag="proj")
                        for dc in range(NDC):
                            nc.tensor.matmul(
                                ps,
                                wk[:, dc, oc * P : (oc + 1) * P],
                                xts[kt][:, dc, :],
                                start=(dc == 0),
                                stop=(dc == NDC - 1),
                            )
                        nc.vector.tensor_copy(kT[:, oc, kt * QT : (kt + 1) * QT], ps)

                # Q projection: qT[o, q] (own 1024 q)
                for qt in range(2):
                    xqt = xsp.tile([P, NDC, QT], bf, tag="xs", name=f"xq{qt}")
                    for dc in range(NDC):
                        sl = slice(qt * QT, (qt + 1) * QT)
                        nc.sync.dma_start(xqt[:, dc, :], xq_r[:, dc, sl])
                    for oc in range(NOC):
                        ps = psA.tile([P, QT], f32, tag="proj")
                        for dc in range(NDC):
                            nc.tensor.matmul(
                                ps,
                                wq[:, dc, oc * P : (oc + 1) * P],
                                xqt[:, dc, :],
                                start=(dc == 0),
                                stop=(dc == NDC - 1),
                            )
                        nc.vector.tensor_copy(qT[:, oc, qt * QT : (qt + 1) * QT], ps)

                # V projection: v[k, o] (full o)
                for kt in range(NQT):
                    xvt = xsp.tile([P, NDC, QT], bf, tag="xs", name=f"xv{kt}")
                    for dc in range(NDC):
                        sl = slice(kt * QT, (kt + 1) * QT)
                        nc.sync.dma_start(xvt[:, dc, :], xT_r[:, dc, sl])
                    for kl in range(4):
                        kc = kt * 4 + kl
                        for oh in range(2):
                            ps = psA.tile([P, QT], f32, tag="proj")
                            for dc in range(NDC):
                                nc.tensor.matmul(
                                    ps,
                                    xvt[:, dc, kl * P : (kl + 1) * P],
                                    wv[:, dc, oh * QT : (oh + 1) * QT],
                                    start=(dc == 0),
                                    stop=(dc == NDC - 1),
                                )
                            nc.vector.tensor_copy(
                                v[:, kc, oh * QT : (oh + 1) * QT], ps
                            )

            # ---- attention ----
            with (
                tc.tile_pool(name="stps", bufs=2, space="PSUM") as stps,
                tc.tile_pool(name="avps", bufs=2, space="PSUM") as avps,
                tc.tile_pool(name="smps", bufs=2, space="PSUM") as smps,
                tc.tile_pool(name="pTp", bufs=2) as pTp,
                tc.tile_pool(name="outst", bufs=4) as outst,
                tc.tile_pool(name="rcpp", bufs=4) as rcpp,
            ):
                for slot in range(2):
                    sheet = pTp.tile([P, NKC, QT], bf, tag="sheet")
                    # scores at q-half (256) granularity: each half only needs
                    # chunks up to its own causal envelope (= AV_ENV[slot][2h+1])
                    for h in range(2):
                        nk = AV_ENV[slot][2 * h + 1]
                        hq = slice(h * (QT // 2), (h + 1) * (QT // 2))
                        for c in range(nk):
                            ps = stps.tile([P, QT // 2], f32, tag="st")
                            for oc in range(NOC):
                                nc.tensor.matmul(
                                    ps,
                                    kT[:, oc, c * P : (c + 1) * P],
                                    qT[:, oc, slot * QT + h * (QT // 2) :
                                       slot * QT + (h + 1) * (QT // 2)],
                                    start=(oc == 0),
                                    stop=(oc == NOC - 1),
                                )
                            nc.scalar.activation(
                                sheet[:, c, hq], ps, Exp, bias=0.0, scale=SCALE
                            )
                            m = slot * 8 + c
                            nc.vector.tensor_mul(
                                sheet[:, c, hq], sheet[:, c, hq], mks[:, m, hq]
                            )
                    for j in range(4):
                        e = AV_ENV[slot][j]
                        av = avps.tile([P, 2, QT], f32, tag="av")
                        sm = smps.tile([P, 1], f32, tag="sm")
                        for c in range(e):
                            psl = sheet[:, c, j * P : (j + 1) * P]
                            nc.tensor.matmul(
                                av[:, 0, :], psl, v[:, c, :QT],
                                start=(c == 0), stop=(c == e - 1),
                            )
                            nc.tensor.matmul(
                                av[:, 1, :], psl, v[:, c, QT:],
                                start=(c == 0), stop=(c == e - 1),
                            )
                            nc.tensor.matmul(
                                sm, psl, ones[:], start=(c == 0), stop=(c == e - 1)
                            )
                        r = rcpp.tile([P, 1], f32, tag="rcp")
                        nc.vector.reciprocal(r[:], sm)
                        ot = outst.tile([P, D], f32, tag="out")
                        nc.vector.tensor_scalar_mul(ot[:, :QT], av[:, 0, :], r[:])
                        nc.vector.tensor_scalar_mul(ot[:, QT:], av[:, 1, :], r[:])
                        nc.sync.dma_start(out_r[:, slot * 4 + j, :], ot[:])

    nc.compile()
    return nc


def _masks_np(role):
    subs = ROLE_SUBTILES[role]
    k = np.arange(P)[:, None]
    q_loc = np.arange(QT)[None, :]
    # original global query index for each local q column, per slot
    qg = []
    for slot in range(2):
        og = np.empty(QT, dtype=np.int64)
        for j in range(4):
            s = subs[slot * 4 + j]
            og[j * P : (j + 1) * P] = s * P + np.arange(P)
        qg.append(og[None, :])
    ms = []
    for c in range(8):
        ms.append(P * c + k <= qg[0])
    for c in range(16):
        ms.append(P * c + k <= qg[1])
    return np.stack(ms).astype(BF16)


def get_module():
    if "nc" not in _CACHE:
        _CACHE["nc"] = _build_module()
    return _CACHE["nc"]


def make_in_maps(x, W_q, W_k, W_v):
    xT = np.ascontiguousarray(
        np.asarray(x, dtype=np.float32).transpose(0, 2, 1)
    ).astype(BF16)
    wqT = np.ascontiguousarray(np.asarray(W_q, dtype=np.float32).T).astype(BF16)
    wkT = np.ascontiguousarray(np.asarray(W_k, dtype=np.float32).T).astype(BF16)
    wvT = np.ascontiguousarray(np.asarray(W_v, dtype=np.float32).T).astype(BF16)
    masks = [_masks_np(r) for r in range(2)]
    in_maps = []
    for c in range(NCORES):
        b, r = c // 2, c % 2
        xq = np.concatenate(
            [xT[b][:, s * P : (s + 1) * P] for s in ROLE_SUBTILES[r]], axis=1
        )
        in_maps.append(
            {
                "xT": xT[b],
                "xTq": np.ascontiguousarray(xq),
                "wqT": wqT,
                "wkT": wkT,
                "wvT": wvT,
                "masks": masks[r],
            }
        )
    return in_maps


def kernel(x, W_q, W_k, W_v):
    from concourse.bass_utils import run_bass_kernel_spmd

    nc = get_module()
    in_maps = make_in_maps(x, W_q, W_k, W_v)
    res = run_bass_kernel_spmd(
        nc,
        in_maps,
        list(range(NCORES)),
        trace=bool(int(os.environ.get("KERNEL_TRACE", "0"))),
    )
    _CACHE["last_result"] = res
    out = np.empty((B, N, D), dtype=np.float32)
    for c in range(NCORES):
        b, r = c // 2, c % 2
        res_out = res.results[c]["out"]
        for i, s in enumerate(ROLE_SUBTILES[r]):
            out[b, s * P : (s + 1) * P, :] = res_out[i * P : (i + 1) * P]
    return out


# revision 22
# speedup vs baseline: 1.3113x; 1.3113x over previous
"""Causal single-head attention (b=4, n=2048, d=1024, fp32) on 8 TRN2 NeuronCores.

Sharding v2 — uniform padded zig-zag q-split. Core c = (batch c//2, role c%2).
Each role owns 8 of the 16 query subtiles of its batch (zig-zag interleaved,
see ROLE_SUBTILES); every core produces out rows for its own 1024 queries
with the FULL 1024 features.

The SPMD program is identical on all cores; the role only changes host-side
data: which columns land in xTq (own queries), the causal masks, and where
host scatters the output rows. Causal work is padded to the elementwise-max
envelope across the two roles (AV_ENV) so both roles run the same instruction
stream; mask data zeroes the padding.

Per core pipeline (all matmuls bf16 -> fp32 PSUM):
  Scores use the algebraic fold S = x (W_q^T W_k) x^T: the host precomputes
  M = W_q^T W_k in fp32 during sharding prep, the kernel computes
  zq[b,q] = M^T xq^T (own 1024 q, replaces BOTH the Q and K projections) and
  contracts sT[k,q] against the resident x^T over b -- no K projection at all.
  v[k,o] = x W_v.T (full o);
  P = exp(sT/32) * mask (no max subtraction; scaled scores are in [-2.6, 2.6]);
  row sums l[q] via ones-matmul; out[q,o] = (P.T-contraction) / l.
"""

import os
import sys

if os.path.isdir("/opt/trn_rl_repo") and "/opt/trn_rl_repo" not in sys.path:
    sys.path.insert(0, "/opt/trn_rl_repo")

import numpy as np
import ml_dtypes

BF16 = ml_dtypes.bfloat16

B, N, D = 4, 2048, 1024
NCORES = 8
P = 128
QT = 512
NQT = N // QT      # 4 orig q tiles
NKC = N // P       # 16 k chunks
NDC = D // P       # 8 d chunks
NOC = D // P       # 8 o chunks
NQ_OWN = 1024      # own queries per core
SCALE = 1.0 / 32.0

# Zig-zag assignment of the 16 query subtiles (128 rows each) to the two
# roles, chosen so the elementwise-max envelope across roles is minimal:
# slot0 = own subtiles drawn from {0..7}, slot1 from {8..15}.
ROLE_SUBTILES = {
    0: (0, 3, 4, 7, 8, 11, 12, 15),
    1: (1, 2, 5, 6, 9, 10, 13, 14),
}
# k-chunk envelope per (slot, subtile position): max over both roles of the
# causally-needed chunk count for the subtile each role places there.
AV_ENV = ((2, 4, 6, 8), (10, 12, 14, 16))

_CACHE = {}


def _build_module():
    from concourse import bacc
    import concourse.tile as tile
    import concourse.mybir as mybir

    bf = mybir.dt.bfloat16
    f32 = mybir.dt.float32
    Exp = mybir.ActivationFunctionType.Exp

    nc = bacc.Bacc("TRN2", target_bir_lowering=False, debug=False, num_devices=NCORES)

    xT_d = nc.dram_tensor("xT", [D, N], bf, kind="ExternalInput")
    xq_d = nc.dram_tensor("xTq", [D, NQ_OWN], bf, kind="ExternalInput")
    m_d = nc.dram_tensor("m", [D, D], bf, kind="ExternalInput")
    wvT_d = nc.dram_tensor("wvT", [D, D], bf, kind="ExternalInput")
    mk_d = nc.dram_tensor("masks", [24, P, QT], bf, kind="ExternalInput")
    out_d = nc.dram_tensor("out", [NQ_OWN, D], f32, kind="ExternalOutput")

    xT_r = xT_d.ap().rearrange("(dc p) n -> p dc n", p=P)
    xq_r = xq_d.ap().rearrange("(dc p) n -> p dc n", p=P)
    m_r = m_d.ap().rearrange("(dc p) o -> p dc o", p=P)
    wv_r = wvT_d.ap().rearrange("(dc p) o -> p dc o", p=P)
    mk_r = mk_d.ap().rearrange("j p q -> p j q")
    out_r = out_d.ap().rearrange("(s p) o -> p s o", p=P)

    with tile.TileContext(nc) as tc:
        with tc.tile_pool(name="pers", bufs=1) as pers:
            zq = pers.tile([P, NDC, NQ_OWN], bf, tag="zq")
            xT = pers.tile([P, NDC, N], bf, tag="xT")
            v = pers.tile([P, NKC, D], bf, tag="v")
            mks = pers.tile([P, 24, QT], bf, tag="masks")
            ones = pers.tile([P, 1], bf, tag="ones")

            nc.vector.memset(ones[:], 1.0)

            # PE pre-warm while the first DMAs land (HAM ramp).
            with tc.tile_pool(name="warm", bufs=1, space="PSUM") as warmps:
                wsrc = pers.tile([P, QT], bf, tag="wsrc")
                nc.vector.memset(wsrc[:], 0.0)
                wps = warmps.tile([P, QT], f32, tag="warm")
                for _ in range(10):
                    nc.tensor.matmul(wps, wsrc[:, :P], wsrc[:], start=True, stop=True)

            # ---- projections (zq, then V) ----
            with (
                tc.tile_pool(name="wp", bufs=1) as wp,
                tc.tile_pool(name="xsp", bufs=2) as xsp,
                tc.tile_pool(name="psA", bufs=4, space="PSUM") as psA,
            ):
                m = wp.tile([P, NDC, D], bf, tag="m")
                wv = wp.tile([P, NDC, D], bf, tag="wv")
                xqts = []
                for qt in range(2):
                    xqts.append(xsp.tile([P, NDC, QT], bf, tag="xq", name=f"xq{qt}"))
                # DMA issue order = consumption order.
                for dc in range(NDC):
                    nc.sync.dma_start(m[:, dc, :], m_r[:, dc, :])
                    nc.sync.dma_start(xqts[0][:, dc, :], xq_r[:, dc, :QT])
                for dc in range(NDC):
                    nc.sync.dma_start(xqts[1][:, dc, :], xq_r[:, dc, QT:])
                for dc in range(NDC):
                    nc.sync.dma_start(xT[:, dc, :], xT_r[:, dc, :])
                for dc in range(NDC):
                    nc.sync.dma_start(wv[:, dc, :], wv_r[:, dc, :])
                nc.sync.dma_start(mks[:], mk_r)

                # zq projection: zq[b, q] = M^T xq^T (own 1024 q)
                for qt in range(2):
                    for bt in range(NDC):
                        ps = psA.tile([P, QT], f32, tag="proj")
                        for dc in range(NDC):
                            nc.tensor.matmul(
                                ps,
                                m[:, dc, bt * P : (bt + 1) * P],
                                xqts[qt][:, dc, :],
                                start=(dc == 0),
                                stop=(dc == NDC - 1),
                            )
                        nc.vector.tensor_copy(zq[:, bt, qt * QT : (qt + 1) * QT], ps)

                # V projection: v[k, o] (full o), x^T resident
                for kc in range(NKC):
                    for oh in range(2):
                        ps = psA.tile([P, QT], f32, tag="proj")
                        for dc in range(NDC):
                            nc.tensor.matmul(
                                ps,
                                xT[:, dc, kc * P : (kc + 1) * P],
                                wv[:, dc, oh * QT : (oh + 1) * QT],
                                start=(dc == 0),
                                stop=(dc == NDC - 1),
                            )
                        nc.vector.tensor_copy(v[:, kc, oh * QT : (oh + 1) * QT], ps)

            # ---- attention ----
            with (
                tc.tile_pool(name="stps", bufs=2, space="PSUM") as stps,
                tc.tile_pool(name="avps", bufs=2, space="PSUM") as avps,
                tc.tile_pool(name="smps", bufs=2, space="PSUM") as smps,
                tc.tile_pool(name="pTp", bufs=2) as pTp,
                tc.tile_pool(name="outst", bufs=4) as outst,
                tc.tile_pool(name="rcpp", bufs=4) as rcpp,
            ):
                for slot in range(2):
                    sheet = pTp.tile([P, NKC, QT], bf, tag="sheet")
                    # scores at q-half (256) granularity: each half only needs
                    # chunks up to its own causal envelope (= AV_ENV[slot][2h+1])
                    for h in range(2):
                        nk = AV_ENV[slot][2 * h + 1]
                        hq = slice(h * (QT // 2), (h + 1) * (QT // 2))
                        for c in range(nk):
                            ps = stps.tile([P, QT // 2], f32, tag="st")
                            for bc in range(NDC):
                                nc.tensor.matmul(
                                    ps,
                                    xT[:, bc, c * P : (c + 1) * P],
                                    zq[:, bc, slot * QT + h * (QT // 2) :
                                       slot * QT + (h + 1) * (QT // 2)],
                                    start=(bc == 0),
                                    stop=(bc == NDC - 1),
                                )
                            nc.scalar.activation(
                                sheet[:, c, hq], ps, Exp, bias=0.0, scale=SCALE
                            )
                            m = slot * 8 + c
                            nc.vector.tensor_mul(
                                sheet[:, c, hq], sheet[:, c, hq], mks[:, m, hq]
                            )
                    for j in range(4):
                        e = AV_ENV[slot][j]
                        av = avps.tile([P, 2, QT], f32, tag="av")
                        sm = smps.tile([P, 1], f32, tag="sm")
                        for c in range(e):
                            psl = sheet[:, c, j * P : (j + 1) * P]
                            nc.tensor.matmul(
                                av[:, 0, :], psl, v[:, c, :QT],
                                start=(c == 0), stop=(c == e - 1),
                            )
                            nc.tensor.matmul(
                                av[:, 1, :], psl, v[:, c, QT:],
                                start=(c == 0), stop=(c == e - 1),
                            )
                            nc.tensor.matmul(
                                sm, psl, ones[:], start=(c == 0), stop=(c == e - 1)
                            )
                        r = rcpp.tile([P, 1], f32, tag="rcp")
                        nc.vector.reciprocal(r[:], sm)
                        ot = outst.tile([P, D], f32, tag="out")
                        nc.vector.tensor_scalar_mul(ot[:, :QT], av[:, 0, :], r[:])
                        nc.vector.tensor_scalar_mul(ot[:, QT:], av[:, 1, :], r[:])
                        nc.sync.dma_start(out_r[:, slot * 4 + j, :], ot[:])

    nc.compile()
    return nc


def _masks_np(role):
    subs = ROLE_SUBTILES[role]
    k = np.arange(P)[:, None]
    q_loc = np.arange(QT)[None, :]
    # original global query index for each local q column, per slot
    qg = []
    for slot in range(2):
        og = np.empty(QT, dtype=np.int64)
        for j in range(4):
            s = subs[slot * 4 + j]
            og[j * P : (j + 1) * P] = s * P + np.arange(P)
        qg.append(og[None, :])
    ms = []
    for c in range(8):
        ms.append(P * c + k <= qg[0])
    for c in range(16):
        ms.append(P * c + k <= qg[1])
    return np.stack(ms).astype(BF16)


def get_module():
    if "nc" not in _CACHE:
        _CACHE["nc"] = _build_module()
    return _CACHE["nc"]


def make_in_maps(x, W_q, W_k, W_v):
    xT = np.ascontiguousarray(
        np.asarray(x, dtype=np.float32).transpose(0, 2, 1)
    ).astype(BF16)
    W_q = np.asarray(W_q, dtype=np.float32)
    W_k = np.asarray(W_k, dtype=np.float32)
    # scores fold: S = x (W_q^T W_k) x^T -- M computed once in fp32
    m = np.ascontiguousarray(W_q.T @ W_k).astype(BF16)
    wvT = np.ascontiguousarray(np.asarray(W_v, dtype=np.float32).T).astype(BF16)
    masks = [_masks_np(r) for r in range(2)]
    in_maps = []
    for c in range(NCORES):
        b, r = c // 2, c % 2
        xq = np.concatenate(
            [xT[b][:, s * P : (s + 1) * P] for s in ROLE_SUBTILES[r]], axis=1
        )
        in_maps.append(
            {
                "xT": xT[b],
                "xTq": np.ascontiguousarray(xq),
                "m": m,
                "wvT": wvT,
                "masks": masks[r],
            }
        )
    return in_maps


def kernel(x, W_q, W_k, W_v):
    from concourse.bass_utils import run_bass_kernel_spmd

    nc = get_module()
    in_maps = make_in_maps(x, W_q, W_k, W_v)
    res = run_bass_kernel_spmd(
        nc,
        in_maps,
        list(range(NCORES)),
        trace=bool(int(os.environ.get("KERNEL_TRACE", "0"))),
    )
    _CACHE["last_result"] = res
    out = np.empty((B, N, D), dtype=np.float32)
    for c in range(NCORES):
        b, r = c // 2, c % 2
        res_out = res.results[c]["out"]
        for i, s in enumerate(ROLE_SUBTILES[r]):
            out[b, s * P : (s + 1) * P, :] = res_out[i * P : (i + 1) * P]
    return out


# revision 25
# speedup vs baseline: 1.3148x; 1.0026x over previous
"""Causal single-head attention (b=4, n=2048, d=1024, fp32) on 8 TRN2 NeuronCores.

Sharding v2 — uniform padded zig-zag q-split. Core c = (batch c//2, role c%2).
Each role owns 8 of the 16 query subtiles of its batch (zig-zag interleaved,
see ROLE_SUBTILES); every core produces out rows for its own 1024 queries
with the FULL 1024 features.

The SPMD program is identical on all cores; the role only changes host-side
data: which columns land in xTq (own queries), the causal masks, and where
host scatters the output rows. Causal work is padded to the elementwise-max
envelope across the two roles (AV_ENV) so both roles run the same instruction
stream; mask data zeroes the padding.

Per core pipeline (all matmuls bf16 -> fp32 PSUM):
  Scores use the algebraic fold S = x (W_q^T W_k) x^T: the host precomputes
  M = W_q^T W_k in fp32 during sharding prep, the kernel computes
  zq[b,q] = M^T xq^T (own 1024 q, replaces BOTH the Q and K projections) and
  contracts sT[k,q] against the resident x^T over b -- no K projection at all.
  v[k,o] = x W_v.T (full o);
  P = exp(sT/32) * mask (no max subtraction; scaled scores are in [-2.6, 2.6]);
  row sums l[q] via ones-matmul; out[q,o] = (P.T-contraction) / l.
"""

import os
import sys

if os.path.isdir("/opt/trn_rl_repo") and "/opt/trn_rl_repo" not in sys.path:
    sys.path.insert(0, "/opt/trn_rl_repo")

import numpy as np
import ml_dtypes

BF16 = ml_dtypes.bfloat16

B, N, D = 4, 2048, 1024
NCORES = 8
P = 128
QT = 512
NQT = N // QT      # 4 orig q tiles
NKC = N // P       # 16 k chunks
NDC = D // P       # 8 d chunks
NOC = D // P       # 8 o chunks
NQ_OWN = 1024      # own queries per core
SCALE = 1.0 / 32.0

# Zig-zag assignment of the 16 query subtiles (128 rows each) to the two
# roles, chosen so the elementwise-max envelope across roles is minimal:
# slot0 = own subtiles drawn from {0..7}, slot1 from {8..15}.
ROLE_SUBTILES = {
    0: (0, 3, 4, 7, 8, 11, 12, 15),
    1: (1, 2, 5, 6, 9, 10, 13, 14),
}
# k-chunk envelope per (slot, subtile position): max over both roles of the
# causally-needed chunk count for the subtile each role places there.
AV_ENV = ((2, 4, 6, 8), (10, 12, 14, 16))

_CACHE = {}


def _build_module():
    from concourse import bacc
    import concourse.tile as tile
    import concourse.mybir as mybir

    bf = mybir.dt.bfloat16
    f32 = mybir.dt.float32
    Exp = mybir.ActivationFunctionType.Exp

    nc = bacc.Bacc("TRN2", target_bir_lowering=False, debug=False, num_devices=NCORES)

    xT_d = nc.dram_tensor("xT", [D, N], bf, kind="ExternalInput")
    xq_d = nc.dram_tensor("xTq", [D, NQ_OWN], bf, kind="ExternalInput")
    m_d = nc.dram_tensor("m", [D, D], bf, kind="ExternalInput")
    wvT_d = nc.dram_tensor("wvT", [D, D], bf, kind="ExternalInput")
    mk_d = nc.dram_tensor("masks", [24, P, QT], bf, kind="ExternalInput")
    out_d = nc.dram_tensor("out", [NQ_OWN, D], f32, kind="ExternalOutput")

    xT_r = xT_d.ap().rearrange("(dc p) n -> p dc n", p=P)
    xq_r = xq_d.ap().rearrange("(dc p) n -> p dc n", p=P)
    m_r = m_d.ap().rearrange("(dc p) o -> p dc o", p=P)
    wv_r = wvT_d.ap().rearrange("(dc p) o -> p dc o", p=P)
    mk_r = mk_d.ap().rearrange("j p q -> p j q")
    out_r = out_d.ap().rearrange("(s p) o -> p s o", p=P)

    with tile.TileContext(nc) as tc:
        with tc.tile_pool(name="pers", bufs=1) as pers:
            zq = pers.tile([P, NDC, NQ_OWN], bf, tag="zq")
            xT = pers.tile([P, NDC, N], bf, tag="xT")
            v = pers.tile([P, NKC, D], bf, tag="v")
            mks = pers.tile([P, 24, QT], bf, tag="masks")
            ones = pers.tile([P, 1], bf, tag="ones")

            nc.vector.memset(ones[:], 1.0)

            # PE pre-warm while the first DMAs land (HAM ramp).
            with tc.tile_pool(name="warm", bufs=1, space="PSUM") as warmps:
                wsrc = pers.tile([P, QT], bf, tag="wsrc")
                nc.vector.memset(wsrc[:], 0.0)
                wps = warmps.tile([P, QT], f32, tag="warm")
                for _ in range(8):
                    nc.tensor.matmul(wps, wsrc[:, :P], wsrc[:], start=True, stop=True)

            # ---- projections (zq, then V) ----
            with (
                tc.tile_pool(name="wp", bufs=1) as wp,
                tc.tile_pool(name="xsp", bufs=2) as xsp,
                tc.tile_pool(name="psA", bufs=4, space="PSUM") as psA,
            ):
                m = wp.tile([P, NDC, D], bf, tag="m")
                wv = wp.tile([P, NDC, D], bf, tag="wv")
                xqts = []
                for qt in range(2):
                    xqts.append(xsp.tile([P, NDC, QT], bf, tag="xq", name=f"xq{qt}"))
                # DMA issue order = consumption order; the first zq psum
                # group only needs the low-b half of m plus xq0.
                for dc in range(NDC):
                    nc.sync.dma_start(m[:, dc, : D // 2], m_r[:, dc, : D // 2])
                    nc.sync.dma_start(xqts[0][:, dc, :], xq_r[:, dc, :QT])
                for dc in range(NDC):
                    nc.sync.dma_start(m[:, dc, D // 2 :], m_r[:, dc, D // 2 :])
                for dc in range(NDC):
                    nc.sync.dma_start(xqts[1][:, dc, :], xq_r[:, dc, QT:])
                for dc in range(NDC):
                    nc.sync.dma_start(xT[:, dc, :], xT_r[:, dc, :])
                for dc in range(NDC):
                    nc.sync.dma_start(wv[:, dc, :], wv_r[:, dc, :])
                nc.sync.dma_start(mks[:], mk_r)

                # zq projection: zq[b, q] = M^T xq^T (own 1024 q)
                for qt in range(2):
                    for bt in range(NDC):
                        ps = psA.tile([P, QT], f32, tag="proj")
                        for dc in range(NDC):
                            nc.tensor.matmul(
                                ps,
                                m[:, dc, bt * P : (bt + 1) * P],
                                xqts[qt][:, dc, :],
                                start=(dc == 0),
                                stop=(dc == NDC - 1),
                            )
                        nc.vector.tensor_copy(zq[:, bt, qt * QT : (qt + 1) * QT], ps)

                # V projection: v[k, o] (full o), x^T resident
                for kc in range(NKC):
                    for oh in range(2):
                        ps = psA.tile([P, QT], f32, tag="proj")
                        for dc in range(NDC):
                            nc.tensor.matmul(
                                ps,
                                xT[:, dc, kc * P : (kc + 1) * P],
                                wv[:, dc, oh * QT : (oh + 1) * QT],
                                start=(dc == 0),
                                stop=(dc == NDC - 1),
                            )
                        nc.vector.tensor_copy(v[:, kc, oh * QT : (oh + 1) * QT], ps)

            # ---- attention ----
            with (
                tc.tile_pool(name="stps", bufs=2, space="PSUM") as stps,
                tc.tile_pool(name="avps", bufs=2, space="PSUM") as avps,
                tc.tile_pool(name="smps", bufs=2, space="PSUM") as smps,
                tc.tile_pool(name="pTp", bufs=2) as pTp,
                tc.tile_pool(name="outst", bufs=4) as outst,
                tc.tile_pool(name="rcpp", bufs=4) as rcpp,
            ):
                for slot in range(2):
                    sheet = pTp.tile([P, NKC, QT], bf, tag="sheet")
                    # scores at q-half (256) granularity: each half only needs
                    # chunks up to its own causal envelope (= AV_ENV[slot][2h+1])
                    for h in range(2):
                        nk = AV_ENV[slot][2 * h + 1]
                        hq = slice(h * (QT // 2), (h + 1) * (QT // 2))
                        for c in range(nk):
                            ps = stps.tile([P, QT // 2], f32, tag="st")
                            for bc in range(NDC):
                                nc.tensor.matmul(
                                    ps,
                                    xT[:, bc, c * P : (c + 1) * P],
                                    zq[:, bc, slot * QT + h * (QT // 2) :
                                       slot * QT + (h + 1) * (QT // 2)],
                                    start=(bc == 0),
                                    stop=(bc == NDC - 1),
                                )
                            nc.scalar.activation(
                                sheet[:, c, hq], ps, Exp, bias=0.0, scale=SCALE
                            )
                            m = slot * 8 + c
                            nc.vector.tensor_mul(
                                sheet[:, c, hq], sheet[:, c, hq], mks[:, m, hq]
                            )
                    for j in range(4):
                        e = AV_ENV[slot][j]
                        av = avps.tile([P, 2, QT], f32, tag="av")
                        sm = smps.tile([P, 1], f32, tag="sm")
                        for c in range(e):
                            psl = sheet[:, c, j * P : (j + 1) * P]
                            nc.tensor.matmul(
                                av[:, 0, :], psl, v[:, c, :QT],
                                start=(c == 0), stop=(c == e - 1),
                            )
                            nc.tensor.matmul(
                                av[:, 1, :], psl, v[:, c, QT:],
                                start=(c == 0), stop=(c == e - 1),
                            )
                            nc.tensor.matmul(
                                sm, psl, ones[:], start=(c == 0), stop=(c == e - 1)
                            )
                        r = rcpp.tile([P, 1], f32, tag="rcp")
                        nc.vector.reciprocal(r[:], sm)
                        ot = outst.tile([P, D], f32, tag="out")
                        s_idx = slot * 4 + j
                        nc.vector.tensor_scalar_mul(ot[:, :QT], av[:, 0, :], r[:])
                        nc.sync.dma_start(out_r[:, s_idx, :QT], ot[:, :QT])
                        nc.vector.tensor_scalar_mul(ot[:, QT:], av[:, 1, :], r[:])
                        nc.sync.dma_start(out_r[:, s_idx, QT:], ot[:, QT:])

    nc.compile()
    return nc


def _masks_np(role):
    subs = ROLE_SUBTILES[role]
    k = np.arange(P)[:, None]
    q_loc = np.arange(QT)[None, :]
    # original global query index for each local q column, per slot
    qg = []
    for slot in range(2):
        og = np.empty(QT, dtype=np.int64)
        for j in range(4):
            s = subs[slot * 4 + j]
            og[j * P : (j + 1) * P] = s * P + np.arange(P)
        qg.append(og[None, :])
    ms = []
    for c in range(8):
        ms.append(P * c + k <= qg[0])
    for c in range(16):
        ms.append(P * c + k <= qg[1])
    return np.stack(ms).astype(BF16)


def get_module():
    if "nc" not in _CACHE:
        _CACHE["nc"] = _build_module()
    return _CACHE["nc"]


def make_in_maps(x, W_q, W_k, W_v):
    xT = np.ascontiguousarray(
        np.asarray(x, dtype=np.float32).transpose(0, 2, 1)
    ).astype(BF16)
    W_q = np.asarray(W_q, dtype=np.float32)
    W_k = np.asarray(W_k, dtype=np.float32)
    # scores fold: S = x (W_q^T W_k) x^T -- M computed once in fp32
    m = np.ascontiguousarray(W_q.T @ W_k).astype(BF16)
    wvT = np.ascontiguousarray(np.asarray(W_v, dtype=np.float32).T).astype(BF16)
    masks = [_masks_np(r) for r in range(2)]
    in_maps = []
    for c in range(NCORES):
        b, r = c // 2, c % 2
        xq = np.concatenate(
            [xT[b][:, s * P : (s + 1) * P] for s in ROLE_SUBTILES[r]], axis=1
        )
        in_maps.append(
            {
                "xT": xT[b],
                "xTq": np.ascontiguousarray(xq),
                "m": m,
                "wvT": wvT,
                "masks": masks[r],
            }
        )
    return in_maps


def kernel(x, W_q, W_k, W_v):
    from concourse.bass_utils import run_bass_kernel_spmd

    nc = get_module()
    in_maps = make_in_maps(x, W_q, W_k, W_v)
    res = run_bass_kernel_spmd(
        nc,
        in_maps,
        list(range(NCORES)),
        trace=bool(int(os.environ.get("KERNEL_TRACE", "0"))),
    )
    _CACHE["last_result"] = res
    out = np.empty((B, N, D), dtype=np.float32)
    for c in range(NCORES):
        b, r = c // 2, c % 2
        res_out = res.results[c]["out"]
        for i, s in enumerate(ROLE_SUBTILES[r]):
            out[b, s * P : (s + 1) * P, :] = res_out[i * P : (i + 1) * P]
    return out


# revision 31
# speedup vs baseline: 1.5249x; 1.1598x over previous
"""Causal single-head attention (b=4, n=2048, d=1024, fp32) on 8 TRN2 NeuronCores.

Sharding v2 — uniform padded zig-zag q-split. Core c = (batch c//2, role c%2).
Each role owns 8 of the 16 query subtiles of its batch (zig-zag interleaved,
see ROLE_SUBTILES); every core produces out rows for its own 1024 queries
with the FULL 1024 features.

The SPMD program is identical on all cores; the role only changes host-side
data: which columns land in xTq (own queries), the causal masks, and where
host scatters the output rows. Causal work is padded to the elementwise-max
envelope across the two roles (AV_ENV) so both roles run the same instruction
stream; mask data zeroes the padding.

Per core pipeline (all matmuls bf16 -> fp32 PSUM):
  Scores use the algebraic fold S = x (W_q^T W_k) x^T: the host precomputes
  M = W_q^T W_k in fp32 during sharding prep, the kernel computes
  zq[b,q] = M^T xq^T (own 1024 q, replaces BOTH the Q and K projections) and
  contracts sT[k,q] against the resident x^T over b -- no K projection at all.
  P = exp(sT/32) * mask (no max subtraction; scaled scores are in [-2.6, 2.6]);
  row sums l[q] via ones-matmul. The value side is folded the same way:
  instead of projecting V = x W_v^T and contracting P V, compute
  U^T[b,q] = x^T P (causal at q-half granularity) and then
  out[q,o] = (U W_v^T) / l -- both contractions run at full causal tightness
  and the V projection disappears.
"""

import os
import sys

if os.path.isdir("/opt/trn_rl_repo") and "/opt/trn_rl_repo" not in sys.path:
    sys.path.insert(0, "/opt/trn_rl_repo")

import numpy as np
import ml_dtypes

BF16 = ml_dtypes.bfloat16

B, N, D = 4, 2048, 1024
NCORES = 8
P = 128
QT = 512
NQT = N // QT      # 4 orig q tiles
NKC = N // P       # 16 k chunks
NDC = D // P       # 8 d chunks
NOC = D // P       # 8 o chunks
NQ_OWN = 1024      # own queries per core
SCALE = 1.0 / 32.0

# Zig-zag assignment of the 16 query subtiles (128 rows each) to the two
# roles, chosen so the elementwise-max envelope across roles is minimal:
# slot0 = own subtiles drawn from {0..7}, slot1 from {8..15}.
ROLE_SUBTILES = {
    0: (0, 3, 4, 7, 8, 11, 12, 15),
    1: (1, 2, 5, 6, 9, 10, 13, 14),
}
# k-chunk envelope per (slot, subtile position): max over both roles of the
# causally-needed chunk count for the subtile each role places there.
AV_ENV = ((2, 4, 6, 8), (10, 12, 14, 16))

_CACHE = {}


def _build_module():
    from concourse import bacc
    import concourse.tile as tile
    import concourse.mybir as mybir

    bf = mybir.dt.bfloat16
    f32 = mybir.dt.float32
    Exp = mybir.ActivationFunctionType.Exp

    nc = bacc.Bacc("TRN2", target_bir_lowering=False, debug=False, num_devices=NCORES)

    xT_d = nc.dram_tensor("xT", [D, N], bf, kind="ExternalInput")
    xN_d = nc.dram_tensor("xN", [N, D], bf, kind="ExternalInput")
    xq_d = nc.dram_tensor("xTq", [D, NQ_OWN], bf, kind="ExternalInput")
    m_d = nc.dram_tensor("m", [D, D], bf, kind="ExternalInput")
    wvT_d = nc.dram_tensor("wvT", [D, D], bf, kind="ExternalInput")
    mk_d = nc.dram_tensor("masks", [24, P, QT], bf, kind="ExternalInput")
    out_d = nc.dram_tensor("out", [NQ_OWN, D], f32, kind="ExternalOutput")

    xT_r = xT_d.ap().rearrange("(dc p) n -> p dc n", p=P)
    xN_r = xN_d.ap().rearrange("(kc p) b -> p kc b", p=P)
    xq_r = xq_d.ap().rearrange("(dc p) n -> p dc n", p=P)
    m_r = m_d.ap().rearrange("(dc p) o -> p dc o", p=P)
    wv_r = wvT_d.ap().rearrange("(dc p) o -> p dc o", p=P)
    mk_r = mk_d.ap().rearrange("j p q -> p j q")
    out_r = out_d.ap().rearrange("(s p) o -> p s o", p=P)

    with tile.TileContext(nc) as tc:
        with tc.tile_pool(name="pers", bufs=1) as pers:
            zq = pers.tile([P, NDC, NQ_OWN], bf, tag="zq")
            xT = pers.tile([P, NDC, N], bf, tag="xT")
            xN = pers.tile([P, NKC, D], bf, tag="xN")
            wv = pers.tile([P, NDC, D], bf, tag="wv")
            mks = pers.tile([P, 24, QT], bf, tag="masks")
            ones = pers.tile([P, 1], bf, tag="ones")

            nc.vector.memset(ones[:], 1.0)

            # PE pre-warm while the first DMAs land (HAM ramp).
            with tc.tile_pool(name="warm", bufs=1, space="PSUM") as warmps:
                wsrc = pers.tile([P, QT], bf, tag="wsrc")
                nc.vector.memset(wsrc[:], 0.0)
                wps = warmps.tile([P, QT], f32, tag="warm")
                for _ in range(8):
                    nc.tensor.matmul(wps, wsrc[:, :P], wsrc[:], start=True, stop=True)

            # ---- zq projection ----
            with (
                tc.tile_pool(name="wp", bufs=1) as wp,
                tc.tile_pool(name="xsp", bufs=2) as xsp,
                tc.tile_pool(name="psA", bufs=4, space="PSUM") as psA,
            ):
                m = wp.tile([P, NDC, D], bf, tag="m")
                xqts = []
                for qt in range(2):
                    xqts.append(xsp.tile([P, NDC, QT], bf, tag="xq", name=f"xq{qt}"))
                # DMA issue order = consumption order; the first zq psum
                # group only needs the low-b half of m plus xq0.
                for dc in range(NDC):
                    nc.sync.dma_start(m[:, dc, : D // 2], m_r[:, dc, : D // 2])
                    nc.sync.dma_start(xqts[0][:, dc, :], xq_r[:, dc, :QT])
                for dc in range(NDC):
                    nc.sync.dma_start(m[:, dc, D // 2 :], m_r[:, dc, D // 2 :])
                for dc in range(NDC):
                    nc.sync.dma_start(xqts[1][:, dc, :], xq_r[:, dc, QT:])
                for dc in range(NDC):
                    nc.sync.dma_start(xT[:, dc, :], xT_r[:, dc, :])
                nc.sync.dma_start(mks[:], mk_r)
                for kc in range(NKC // 2):
                    nc.sync.dma_start(xN[:, kc, :], xN_r[:, kc, :])
                for dc in range(NDC):
                    nc.sync.dma_start(wv[:, dc, :], wv_r[:, dc, :])
                for kc in range(NKC // 2, NKC):
                    nc.sync.dma_start(xN[:, kc, :], xN_r[:, kc, :])

                # zq projection: zq[b, q] = M^T xq^T (own 1024 q)
                for qt in range(2):
                    for bt in range(NDC):
                        ps = psA.tile([P, QT], f32, tag="proj")
                        for dc in range(NDC):
                            nc.tensor.matmul(
                                ps,
                                m[:, dc, bt * P : (bt + 1) * P],
                                xqts[qt][:, dc, :],
                                start=(dc == 0),
                                stop=(dc == NDC - 1),
                            )
                        nc.vector.tensor_copy(zq[:, bt, qt * QT : (qt + 1) * QT], ps)

            # ---- attention ----
            HQ = QT // 2
            with (
                tc.tile_pool(name="stps", bufs=2, space="PSUM") as stps,
                tc.tile_pool(name="smps", bufs=2, space="PSUM") as smps,
                tc.tile_pool(name="psU", bufs=2, space="PSUM") as psU,
                tc.tile_pool(name="outp", bufs=2, space="PSUM") as outp,
                tc.tile_pool(name="pTp", bufs=2) as pTp,
                tc.tile_pool(name="uTp", bufs=1) as uTp,
                tc.tile_pool(name="outst", bufs=2) as outst,
                tc.tile_pool(name="rcpp", bufs=8) as rcpp,
            ):
                uT = uTp.tile([P, NDC, NQ_OWN], bf, tag="uT")
                for slot in range(2):
                    sheet = pTp.tile([P, NKC, QT], bf, tag="sheet")
                    # scores at q-half (256) granularity: each half only needs
                    # chunks up to its own causal envelope (= AV_ENV[slot][2h+1])
                    for h in range(2):
                        nk = AV_ENV[slot][2 * h + 1]
                        hq = slice(h * HQ, (h + 1) * HQ)
                        for c in range(nk):
                            ps = stps.tile([P, HQ], f32, tag="st")
                            for bc in range(NDC):
                                nc.tensor.matmul(
                                    ps,
                                    xT[:, bc, c * P : (c + 1) * P],
                                    zq[:, bc, slot * QT + h * HQ :
                                       slot * QT + (h + 1) * HQ],
                                    start=(bc == 0),
                                    stop=(bc == NDC - 1),
                                )
                            nc.scalar.activation(
                                sheet[:, c, hq], ps, Exp, bias=0.0, scale=SCALE
                            )
                            mi = slot * 8 + c
                            nc.vector.tensor_mul(
                                sheet[:, c, hq], sheet[:, c, hq], mks[:, mi, hq]
                            )
                    # softmax row sums per subtile position
                    rs = []
                    for j in range(4):
                        e = AV_ENV[slot][j]
                        sm = smps.tile([P, 1], f32, tag="sm")
                        for c in range(e):
                            nc.tensor.matmul(
                                sm, sheet[:, c, j * P : (j + 1) * P], ones[:],
                                start=(c == 0), stop=(c == e - 1),
                            )
                        r = rcpp.tile([P, 1], f32, tag="rcp", name=f"r{slot}{j}")
                        nc.vector.reciprocal(r[:], sm)
                        rs.append(r)
                    # U^T[b, q] = x^T P (causal at q-half granularity)
                    for h in range(2):
                        nk = AV_ENV[slot][2 * h + 1]
                        hq = slice(h * HQ, (h + 1) * HQ)
                        for bt in range(NDC):
                            ps = psU.tile([P, HQ], f32, tag="ut")
                            for c in range(nk):
                                nc.tensor.matmul(
                                    ps,
                                    xN[:, c, bt * P : (bt + 1) * P],
                                    sheet[:, c, hq],
                                    start=(c == 0),
                                    stop=(c == nk - 1),
                                )
                            nc.vector.tensor_copy(
                                uT[:, bt, slot * QT + h * HQ :
                                   slot * QT + (h + 1) * HQ],
                                ps,
                            )
                    # out[q, o] = U Wv^T, normalized by the row sums
                    for j in range(4):
                        s_idx = slot * 4 + j
                        qs = slice(slot * QT + j * P, slot * QT + (j + 1) * P)
                        for oh in range(2):
                            ps = outp.tile([P, QT], f32, tag="out")
                            for bc in range(NDC):
                                nc.tensor.matmul(
                                    ps,
                                    uT[:, bc, qs],
                                    wv[:, bc, oh * QT : (oh + 1) * QT],
                                    start=(bc == 0),
                                    stop=(bc == NDC - 1),
                                )
                            ot = outst.tile([P, QT], f32, tag="ot")
                            nc.vector.tensor_scalar_mul(ot[:], ps, rs[j][:])
                            nc.sync.dma_start(
                                out_r[:, s_idx, oh * QT : (oh + 1) * QT], ot[:]
                            )

    nc.compile()
    return nc


def _masks_np(role):
    subs = ROLE_SUBTILES[role]
    k = np.arange(P)[:, None]
    q_loc = np.arange(QT)[None, :]
    # original global query index for each local q column, per slot
    qg = []
    for slot in range(2):
        og = np.empty(QT, dtype=np.int64)
        for j in range(4):
            s = subs[slot * 4 + j]
            og[j * P : (j + 1) * P] = s * P + np.arange(P)
        qg.append(og[None, :])
    ms = []
    for c in range(8):
        ms.append(P * c + k <= qg[0])
    for c in range(16):
        ms.append(P * c + k <= qg[1])
    return np.stack(ms).astype(BF16)


def get_module():
    if "nc" not in _CACHE:
        _CACHE["nc"] = _build_module()
    return _CACHE["nc"]


def make_in_maps(x, W_q, W_k, W_v):
    xT = np.ascontiguousarray(
        np.asarray(x, dtype=np.float32).transpose(0, 2, 1)
    ).astype(BF16)
    W_q = np.asarray(W_q, dtype=np.float32)
    W_k = np.asarray(W_k, dtype=np.float32)
    # scores fold: S = x (W_q^T W_k) x^T -- M computed once in fp32
    m = np.ascontiguousarray(W_q.T @ W_k).astype(BF16)
    wvT = np.ascontiguousarray(np.asarray(W_v, dtype=np.float32).T).astype(BF16)
    masks = [_masks_np(r) for r in range(2)]
    in_maps = []
    for c in range(NCORES):
        b, r = c // 2, c % 2
        xq = np.concatenate(
            [xT[b][:, s * P : (s + 1) * P] for s in ROLE_SUBTILES[r]], axis=1
        )
        in_maps.append(
            {
                "xT": xT[b],
                "xN": np.ascontiguousarray(xT[b].T),
                "xTq": np.ascontiguousarray(xq),
                "m": m,
                "wvT": wvT,
                "masks": masks[r],
            }
        )
    return in_maps


def kernel(x, W_q, W_k, W_v):
    from concourse.bass_utils import run_bass_kernel_spmd

    nc = get_module()
    in_maps = make_in_maps(x, W_q, W_k, W_v)
    res = run_bass_kernel_spmd(
        nc,
        in_maps,
        list(range(NCORES)),
        trace=bool(int(os.environ.get("KERNEL_TRACE", "0"))),
    )
    _CACHE["last_result"] = res
    out = np.empty((B, N, D), dtype=np.float32)
    for c in range(NCORES):
        b, r = c // 2, c % 2
        res_out = res.results[c]["out"]
        for i, s in enumerate(ROLE_SUBTILES[r]):
            out[b, s * P : (s + 1) * P, :] = res_out[i * P : (i + 1) * P]
    return out


# revision 32
# speedup vs baseline: 1.5326x; 1.0050x over previous
"""Causal single-head attention (b=4, n=2048, d=1024, fp32) on 8 TRN2 NeuronCores.

Sharding v2 — uniform padded zig-zag q-split. Core c = (batch c//2, role c%2).
Each role owns 8 of the 16 query subtiles of its batch (zig-zag interleaved,
see ROLE_SUBTILES); every core produces out rows for its own 1024 queries
with the FULL 1024 features.

The SPMD program is identical on all cores; the role only changes host-side
data: which columns land in xTq (own queries), the causal masks, and where
host scatters the output rows. Causal work is padded to the elementwise-max
envelope across the two roles (AV_ENV) so both roles run the same instruction
stream; mask data zeroes the padding.

Per core pipeline (all matmuls bf16 -> fp32 PSUM):
  Scores use the algebraic fold S = x (W_q^T W_k) x^T: the host precomputes
  M = W_q^T W_k in fp32 during sharding prep, the kernel computes
  zq[b,q] = M^T xq^T (own 1024 q, replaces BOTH the Q and K projections) and
  contracts sT[k,q] against the resident x^T over b -- no K projection at all.
  P = exp(sT/32) * mask (no max subtraction; scaled scores are in [-2.6, 2.6]);
  row sums l[q] via ones-matmul. The value side is folded the same way:
  instead of projecting V = x W_v^T and contracting P V, compute
  U^T[b,q] = x^T P (causal at q-half granularity) and then
  out[q,o] = (U W_v^T) / l -- both contractions run at full causal tightness
  and the V projection disappears.
"""

import os
import sys

if os.path.isdir("/opt/trn_rl_repo") and "/opt/trn_rl_repo" not in sys.path:
    sys.path.insert(0, "/opt/trn_rl_repo")

import numpy as np
import ml_dtypes

BF16 = ml_dtypes.bfloat16

B, N, D = 4, 2048, 1024
NCORES = 8
P = 128
QT = 512
NQT = N // QT      # 4 orig q tiles
NKC = N // P       # 16 k chunks
NDC = D // P       # 8 d chunks
NOC = D // P       # 8 o chunks
NQ_OWN = 1024      # own queries per core
SCALE = 1.0 / 32.0

# Zig-zag assignment of the 16 query subtiles (128 rows each) to the two
# roles, chosen so the elementwise-max envelope across roles is minimal:
# slot0 = own subtiles drawn from {0..7}, slot1 from {8..15}.
ROLE_SUBTILES = {
    0: (0, 3, 4, 7, 8, 11, 12, 15),
    1: (1, 2, 5, 6, 9, 10, 13, 14),
}
# k-chunk envelope per (slot, subtile position): max over both roles of the
# causally-needed chunk count for the subtile each role places there.
AV_ENV = ((2, 4, 6, 8), (10, 12, 14, 16))

_CACHE = {}


def _build_module():
    from concourse import bacc
    import concourse.tile as tile
    import concourse.mybir as mybir

    bf = mybir.dt.bfloat16
    f32 = mybir.dt.float32
    Exp = mybir.ActivationFunctionType.Exp

    nc = bacc.Bacc("TRN2", target_bir_lowering=False, debug=False, num_devices=NCORES)

    xT_d = nc.dram_tensor("xT", [D, N], bf, kind="ExternalInput")
    xN_d = nc.dram_tensor("xN", [N, D], bf, kind="ExternalInput")
    xq_d = nc.dram_tensor("xTq", [D, NQ_OWN], bf, kind="ExternalInput")
    m_d = nc.dram_tensor("m", [D, D], bf, kind="ExternalInput")
    wvT_d = nc.dram_tensor("wvT", [D, D], bf, kind="ExternalInput")
    mk_d = nc.dram_tensor("masks", [24, P, QT], bf, kind="ExternalInput")
    out_d = nc.dram_tensor("out", [NQ_OWN, D], f32, kind="ExternalOutput")

    xT_r = xT_d.ap().rearrange("(dc p) n -> p dc n", p=P)
    xN_r = xN_d.ap().rearrange("(kc p) b -> p kc b", p=P)
    xq_r = xq_d.ap().rearrange("(dc p) n -> p dc n", p=P)
    m_r = m_d.ap().rearrange("(dc p) o -> p dc o", p=P)
    wv_r = wvT_d.ap().rearrange("(dc p) o -> p dc o", p=P)
    mk_r = mk_d.ap().rearrange("j p q -> p j q")
    out_r = out_d.ap().rearrange("(s p) o -> p s o", p=P)

    with tile.TileContext(nc) as tc:
        with tc.tile_pool(name="pers", bufs=1) as pers:
            zq = pers.tile([P, NDC, NQ_OWN], bf, tag="zq")
            xT = pers.tile([P, NDC, N], bf, tag="xT")
            xN = pers.tile([P, NKC, D], bf, tag="xN")
            wv = pers.tile([P, NDC, D], bf, tag="wv")
            mks = pers.tile([P, 24, QT], bf, tag="masks")
            ones = pers.tile([P, 1], bf, tag="ones")

            nc.vector.memset(ones[:], 1.0)

            # PE pre-warm while the first DMAs land (HAM ramp).
            with tc.tile_pool(name="warm", bufs=1, space="PSUM") as warmps:
                wsrc = pers.tile([P, QT], bf, tag="wsrc")
                nc.vector.memset(wsrc[:], 0.0)
                wps = warmps.tile([P, QT], f32, tag="warm")
                for _ in range(8):
                    nc.tensor.matmul(wps, wsrc[:, :P], wsrc[:], start=True, stop=True)

            # ---- zq projection ----
            with (
                tc.tile_pool(name="wp", bufs=1) as wp,
                tc.tile_pool(name="xsp", bufs=2) as xsp,
                tc.tile_pool(name="psA", bufs=4, space="PSUM") as psA,
            ):
                m = wp.tile([P, NDC, D], bf, tag="m")
                xqts = []
                for qt in range(2):
                    xqts.append(xsp.tile([P, NDC, QT], bf, tag="xq", name=f"xq{qt}"))
                # DMA issue order = consumption order; the first zq psum
                # group only needs the low-b half of m plus xq0.
                for dc in range(NDC):
                    nc.sync.dma_start(m[:, dc, : D // 2], m_r[:, dc, : D // 2])
                    nc.sync.dma_start(xqts[0][:, dc, :], xq_r[:, dc, :QT])
                for dc in range(NDC):
                    nc.sync.dma_start(m[:, dc, D // 2 :], m_r[:, dc, D // 2 :])
                for dc in range(NDC):
                    nc.sync.dma_start(xqts[1][:, dc, :], xq_r[:, dc, QT:])
                for dc in range(NDC):
                    nc.sync.dma_start(xT[:, dc, :], xT_r[:, dc, :])
                nc.sync.dma_start(mks[:], mk_r)
                for kc in range(NKC // 2):
                    nc.sync.dma_start(xN[:, kc, :], xN_r[:, kc, :])
                for dc in range(NDC):
                    nc.sync.dma_start(wv[:, dc, :], wv_r[:, dc, :])
                for kc in range(NKC // 2, NKC):
                    nc.sync.dma_start(xN[:, kc, :], xN_r[:, kc, :])

                # zq projection: zq[b, q] = M^T xq^T (own 1024 q)
                for qt in range(2):
                    for bt in range(NDC):
                        ps = psA.tile([P, QT], f32, tag="proj")
                        for dc in range(NDC):
                            nc.tensor.matmul(
                                ps,
                                m[:, dc, bt * P : (bt + 1) * P],
                                xqts[qt][:, dc, :],
                                start=(dc == 0),
                                stop=(dc == NDC - 1),
                            )
                        nc.vector.tensor_copy(zq[:, bt, qt * QT : (qt + 1) * QT], ps)

            # ---- attention ----
            HQ = QT // 2
            with (
                tc.tile_pool(name="stps", bufs=2, space="PSUM") as stps,
                tc.tile_pool(name="smps", bufs=2, space="PSUM") as smps,
                tc.tile_pool(name="psU", bufs=2, space="PSUM") as psU,
                tc.tile_pool(name="outp", bufs=2, space="PSUM") as outp,
                tc.tile_pool(name="pTp", bufs=2) as pTp,
                tc.tile_pool(name="uTp", bufs=1) as uTp,
                tc.tile_pool(name="outst", bufs=2) as outst,
                tc.tile_pool(name="rcpp", bufs=8) as rcpp,
            ):
                uT = uTp.tile([P, NDC, NQ_OWN], bf, tag="uT")
                for slot in range(2):
                    sheet = pTp.tile([P, NKC, QT], bf, tag="sheet")
                    # scores at q-half (256) granularity: each half only needs
                    # chunks up to its own causal envelope (= AV_ENV[slot][2h+1])
                    for h in range(2):
                        nk = AV_ENV[slot][2 * h + 1]
                        hq = slice(h * HQ, (h + 1) * HQ)
                        for c in range(nk):
                            ps = stps.tile([P, HQ], f32, tag="st")
                            for bc in range(NDC):
                                nc.tensor.matmul(
                                    ps,
                                    xT[:, bc, c * P : (c + 1) * P],
                                    zq[:, bc, slot * QT + h * HQ :
                                       slot * QT + (h + 1) * HQ],
                                    start=(bc == 0),
                                    stop=(bc == NDC - 1),
                                )
                            nc.scalar.activation(
                                sheet[:, c, hq], ps, Exp, bias=0.0, scale=SCALE
                            )
                            mi = slot * 8 + c
                            nc.vector.tensor_mul(
                                sheet[:, c, hq], sheet[:, c, hq], mks[:, mi, hq]
                            )
                    # Per half: U^T (fills the ACT exp/mask lag of the other
                    # half), then this half's row sums and output columns.
                    for h in range(2):
                        nk = AV_ENV[slot][2 * h + 1]
                        hq = slice(h * HQ, (h + 1) * HQ)
                        # U^T[b, q] = x^T P (causal at q-half granularity)
                        for bt in range(NDC):
                            ps = psU.tile([P, HQ], f32, tag="ut")
                            for c in range(nk):
                                nc.tensor.matmul(
                                    ps,
                                    xN[:, c, bt * P : (bt + 1) * P],
                                    sheet[:, c, hq],
                                    start=(c == 0),
                                    stop=(c == nk - 1),
                                )
                            nc.vector.tensor_copy(
                                uT[:, bt, slot * QT + h * HQ :
                                   slot * QT + (h + 1) * HQ],
                                ps,
                            )
                        for j in (2 * h, 2 * h + 1):
                            e = AV_ENV[slot][j]
                            sm = smps.tile([P, 1], f32, tag="sm")
                            for c in range(e):
                                nc.tensor.matmul(
                                    sm, sheet[:, c, j * P : (j + 1) * P], ones[:],
                                    start=(c == 0), stop=(c == e - 1),
                                )
                            r = rcpp.tile([P, 1], f32, tag="rcp", name=f"r{slot}{j}")
                            nc.vector.reciprocal(r[:], sm)
                            # out[q, o] = U Wv^T, normalized by the row sums
                            s_idx = slot * 4 + j
                            qs = slice(slot * QT + j * P, slot * QT + (j + 1) * P)
                            for oh in range(2):
                                ps = outp.tile([P, QT], f32, tag="out")
                                for bc in range(NDC):
                                    nc.tensor.matmul(
                                        ps,
                                        uT[:, bc, qs],
                                        wv[:, bc, oh * QT : (oh + 1) * QT],
                                        start=(bc == 0),
                                        stop=(bc == NDC - 1),
                                    )
                                ot = outst.tile([P, QT], f32, tag="ot")
                                if oh == 0:
                                    nc.scalar.mul(ot[:], ps, r[:])
                                else:
                                    nc.vector.tensor_scalar_mul(ot[:], ps, r[:])
                                nc.sync.dma_start(
                                    out_r[:, s_idx, oh * QT : (oh + 1) * QT], ot[:]
                                )

    nc.compile()
    return nc


def _masks_np(role):
    subs = ROLE_SUBTILES[role]
    k = np.arange(P)[:, None]
    q_loc = np.arange(QT)[None, :]
    # original global query index for each local q column, per slot
    qg = []
    for slot in range(2):
        og = np.empty(QT, dtype=np.int64)
        for j in range(4):
            s = subs[slot * 4 + j]
            og[j * P : (j + 1) * P] = s * P + np.arange(P)
        qg.append(og[None, :])
    ms = []
    for c in range(8):
        ms.append(P * c + k <= qg[0])
    for c in range(16):
        ms.append(P * c + k <= qg[1])
    return np.stack(ms).astype(BF16)


def get_module():
    if "nc" not in _CACHE:
        _CACHE["nc"] = _build_module()
    return _CACHE["nc"]


def make_in_maps(x, W_q, W_k, W_v):
    xT = np.ascontiguousarray(
        np.asarray(x, dtype=np.float32).transpose(0, 2, 1)
    ).astype(BF16)
    W_q = np.asarray(W_q, dtype=np.float32)
    W_k = np.asarray(W_k, dtype=np.float32)
    # scores fold: S = x (W_q^T W_k) x^T -- M computed once in fp32
    m = np.ascontiguousarray(W_q.T @ W_k).astype(BF16)
    wvT = np.ascontiguousarray(np.asarray(W_v, dtype=np.float32).T).astype(BF16)
    masks = [_masks_np(r) for r in range(2)]
    in_maps = []
    for c in range(NCORES):
        b, r = c // 2, c % 2
        xq = np.concatenate(
            [xT[b][:, s * P : (s + 1) * P] for s in ROLE_SUBTILES[r]], axis=1
        )
        in_maps.append(
            {
                "xT": xT[b],
                "xN": np.ascontiguousarray(xT[b].T),
                "xTq": np.ascontiguousarray(xq),
                "m": m,
                "wvT": wvT,
                "masks": masks[r],
            }
        )
    return in_maps


def kernel(x, W_q, W_k, W_v):
    from concourse.bass_utils import run_bass_kernel_spmd

    nc = get_module()
    in_maps = make_in_maps(x, W_q, W_k, W_v)
    res = run_bass_kernel_spmd(
        nc,
        in_maps,
        list(range(NCORES)),
        trace=bool(int(os.environ.get("KERNEL_TRACE", "0"))),
    )
    _CACHE["last_result"] = res
    out = np.empty((B, N, D), dtype=np.float32)
    for c in range(NCORES):
        b, r = c // 2, c % 2
        res_out = res.results[c]["out"]
        for i, s in enumerate(ROLE_SUBTILES[r]):
            out[b, s * P : (s + 1) * P, :] = res_out[i * P : (i + 1) * P]
    return out
